# revision 1
# baseline (speedup 1.0000x reference)
"""Trainium2 Bass kernel for nn_LocalMQA (S=2048, D_MODEL=1024, H=16, D=64, WIN=128).

Sharding: sequence-parallel across 8 cores (256 output rows each) with a
128-row halo recomputed for k/v. No collectives; each core produces a
disjoint slice of the output.

Per-core pipeline (all layouts transposed: feature dim on partitions):
  qkvT = W1T.T @ xT            (fp32r matmuls, fp32 PSUM)
  scores = maskinject + qT.k   (mask via fp16 identity matmul into PSUM,
                                q pre-scaled by sqrt(D) on host)
  softmax: DVE rowmax(negate) -> ACT Exp(bias=-max, accum=rowsum) -> fp16
  attnT via DMA transpose; o = attnT.T @ v16 (fp16) ; normalize by 1/rowsum
  oT via PE transpose; outT = W2T.T @ oT (fp16) + bout  -> DRAM [1024, 256]
Host transposes/concats the 8 outT slices into the final (2048, 1024).
"""
import numpy as np

import concourse.bacc as bacc
import concourse.bass as bass
import concourse.mybir as mybir
import concourse.tile as tile
from concourse.tile_rust import add_dep_helper
from concourse.bass_utils import run_bass_kernel_spmd

S = 2048
DM = 1024
H = 16
D = 64
WIN = 128
NC = 8
RPC = S // NC          # rows per core = 256
HALO = 128
XW = RPC + HALO        # per-core xT width = 384

F32 = mybir.dt.float32
F32R = mybir.dt.float32r
F16 = mybir.dt.float16

QKV_F32R = [True]  # fp32r only in the (PE-isolated) qkv phase

_CACHED = {}

import contextlib
def _nullctx():
    return contextlib.nullcontext()


def _rnd_fp32r(a):
    """Round fp32 to E8M11 (fp32r), round-to-nearest-even — matches PE rounding."""
    u = np.ascontiguousarray(a, dtype=np.float32).view(np.uint32)
    b = ((u >> 12) & 1).astype(np.uint32) + np.uint32((1 << 11) - 1)
    return ((u + b) & np.uint32(0xFFFFF000)).view(np.float32)


def _build(debug=False, reps=None, abl=None):
    PROJ = F32R if QKV_F32R[0] else F32
    nc = bacc.Bacc("TRN2", target_bir_lowering=False, debug=False, num_devices=NC)

    xT_d = nc.dram_tensor("xT", [8, 128, XW], PROJ, kind="ExternalInput").ap()
    w1_d = nc.dram_tensor("w1T", [8, 128, 1152], PROJ, kind="ExternalInput").ap()
    b1_d = nc.dram_tensor("b1", [128, 9], F32, kind="ExternalInput").ap()
    w2_d = nc.dram_tensor("w2T", [8, 128, 1024], F16, kind="ExternalInput").ap()
    b2_d = nc.dram_tensor("b2", [128, 8], F32, kind="ExternalInput").ap()
    msk_d = nc.dram_tensor("mask", [128, 2, 512], F16, kind="ExternalInput").ap()
    id_d = nc.dram_tensor("ident", [128, 128], F16, kind="ExternalInput").ap()
    out_d = nc.dram_tensor("outT", [8, 128, RPC], F32, kind="ExternalOutput").ap()
    id16_d = nc.dram_tensor("ident16", [128, 128], F16, kind="ExternalInput").ap()
    if debug:
        dbg = {
            "dbg_kv": nc.dram_tensor("dbg_kv", [128, XW], F32, kind="ExternalOutput").ap(),
            "dbg_q": nc.dram_tensor("dbg_q", [128, 8, RPC], F32, kind="ExternalOutput").ap(),
            "dbg_v16": nc.dram_tensor("dbg_v16", [128, 3, 64], F32, kind="ExternalOutput").ap(),
            "dbg_attn": nc.dram_tensor("dbg_attn", [128, 256], F32, kind="ExternalOutput").ap(),
            "dbg_negm": nc.dram_tensor("dbg_negm", [128, 1], F32, kind="ExternalOutput").ap(),
            "dbg_rowsum": nc.dram_tensor("dbg_rowsum", [128, 1], F32, kind="ExternalOutput").ap(),
            "dbg_o16": nc.dram_tensor("dbg_o16", [128, 16, 64], F32, kind="ExternalOutput").ap(),
            "dbg_oT": nc.dram_tensor("dbg_oT", [128, 8, 256], F32, kind="ExternalOutput").ap(),
        }

    with tile.TileContext(nc) as tc:
      with (
        tc.tile_pool(name="w", bufs=1) as wp,      # weights + constants
        tc.tile_pool(name="act", bufs=1) as ap_,   # persistent activations
        tc.tile_pool(name="sm", bufs=8) as smp,    # small softmax tiles
        tc.tile_pool(name="att", bufs=6) as attp,
        tc.tile_pool(name="o16p", bufs=1) as o16p,
        tc.tile_pool(name="outp", bufs=2) as outp,
      ):
       with (tc.For_i(0, reps, 1) if reps else _nullctx()):
         with (
             tc.tile_pool(name="ps_q", bufs=2, space="PSUM") as ps_q,
             tc.tile_pool(name="ps_kv", bufs=1, space="PSUM") as ps_kv,
         ):
             # ---- input DMAs ----
             xT = wp.tile([128, 8, XW], PROJ)
             w1 = wp.tile([128, 8, 1152], PROJ)
             w2 = wp.tile([128, 8, 1024], F16)
             b1 = wp.tile([128, 9], F32)
             b2 = wp.tile([128, 8], F32)
             msk2 = wp.tile([128, 2, 512], F16)
             ident = wp.tile([128, 128], F16)
             id16 = wp.tile([128, 128], F16)
             nc.sync.dma_start(id16[:], id16_d)
             for c in range(8):
                 nc.sync.dma_start(xT[:, c, :], xT_d[c])
             for c in range(8):
                 nc.sync.dma_start(w1[:, c, :], w1_d[c])
             nc.sync.dma_start(b1[:], b1_d)
             nc.sync.dma_start(b2[:], b2_d)
             nc.sync.dma_start(msk2[:], msk_d)
             nc.sync.dma_start(ident[:], id_d)
             for c in range(8):
                 nc.sync.dma_start(w2[:, c, :], w2_d[c])

             # ---- qkv projection: qkvT tiles [outdim-part, rows-free] ----
             kv_sb = ap_.tile([128, XW], F32)      # k rows 0:64, v rows 64:128
             v16r = ap_.tile([128, 3, 128], F16)    # v (cast) at partitions 64:128
             q_sb = ap_.tile([128, 8, RPC], F32)    # q tiles, 2 heads per tile

             kvp = ps_kv.tile([128, XW], F32)
             for c in range(8):
                 nc.tensor.matmul(kvp[:], w1[:, c, 0:128], xT[:, c, :],
                                  start=(c == 0), stop=(c == 7))
             # k evac (f32r) on ACT, v evac (f16) on DVE
             nc.scalar.activation(kv_sb[0:64, :], kvp[0:64, :],
                                  mybir.ActivationFunctionType.Identity,
                                  bias=b1[0:64, 0:1], scale=1.0)
             nc.vector.tensor_scalar_add(
                 v16r[64:128, :, :].rearrange("p b n -> p (b n)"), kvp[64:128, :],
                 b1[64:128, 0:1])

             last_qkv = None
             for t in range(8):
                 qp = ps_q.tile([128, RPC], F32)
                 for c in range(8):
                     last_qkv = nc.tensor.matmul(
                         qp[:], w1[:, c, 128 * (t + 1):128 * (t + 2)],
                         xT[:, c, HALO:XW],
                         start=(c == 0), stop=(c == 7))
                 nc.vector.tensor_scalar_add(q_sb[:, t, :], qp[:], b1[:, t + 1:t + 2])

             # v16: transpose v [64, 384] -> 3 blocks [128, 64] fp16 via DMA
             v16 = ap_.tile([128, 3, 64], F16)
             for b in range(3):
                 nc.sync.dma_start(v16[:, b, :], v16r[64:128, b, :], transpose=True)
             # mirror k into partitions 64:128 so odd heads (q at base
             # partition 64) have a same-base rhs (matmul requirement)
             nc.sync.dma_start(kv_sb[64:128, :], kv_sb[0:64, :])
             if debug in (True, "phase", "kv"):
                 nc.sync.dma_start(dbg["dbg_kv"], kv_sb[:].bitcast(F32))
             if debug in (True, "phase", "q"):
                 nc.sync.dma_start(dbg["dbg_q"], q_sb[:].bitcast(F32))
             if debug in (True, "phase", "v"):
                 dv = ap_.tile([128, 3, 64], F32, tag="dbgv")
                 nc.vector.tensor_copy(dv[:], v16[:])
                 nc.sync.dma_start(dbg["dbg_v16"], dv[:])

         with (
             tc.tile_pool(name="ps_s", bufs=3, space="PSUM") as ps_s,
             tc.tile_pool(name="ps_o", bufs=1, space="PSUM") as ps_o,
             tc.tile_pool(name="ps_t", bufs=2, space="PSUM") as ps_t,
             tc.tile_pool(name="ps_f", bufs=2, space="PSUM") as ps_f,
         ):
             oT_sb = o16p.tile([128, 8, 2 * 128], F16)  # [hd-chunk, itile*128+i]
             for it in range(2):
                 o16 = o16p.tile([128, 16, 64], F16, tag=f"o16_{it}")
                 if abl == "noatt":
                     nc.vector.memset(o16[:].rearrange("p a b -> p (a b)"), 0.0)
                 for h in ([] if abl == "noatt" else range(16)):
                     sc = ps_s.tile([128, 256], F32)
                     inj = nc.tensor.matmul(sc[:], id16[:],
                                            msk2[:, min(it, 1), 0:256],
                                            start=True, stop=False)
                     add_dep_helper(inj.ins, last_qkv.ins, sync=True,
                                    reason="PE gate: f32r qkv before f16 mms")
                     nc.tensor.matmul(
                         sc[:],
                         q_sb[64 * (h % 2):64 * (h % 2) + 64, h // 2,
                              it * 128:it * 128 + 128],
                         kv_sb[64 * (h % 2):64 * (h % 2) + 64,
                               it * 128:it * 128 + 256],
                         start=False, stop=True)
                     negm = smp.tile([128, 1], F32, tag="negm")
                     nc.vector.tensor_reduce(negm[:], sc[:],
                                             axis=mybir.AxisListType.X,
                                             op=mybir.AluOpType.max, negate=True)
                     rowsum = smp.tile([128, 1], F32, tag="rowsum")
                     attn = attp.tile([128, 256], F16, tag="attn")
                     nc.scalar.activation(attn[:], sc[:],
                                          mybir.ActivationFunctionType.Exp,
                                          bias=negm[:], scale=1.0,
                                          accum_out=rowsum[:])
                     attnT = attp.tile([128, 2, 128], F16, tag="attnT")
                     for b in range(2):
                         ptt = ps_t.tile([128, 128], F16, tag="tp")
                         nc.tensor.transpose(ptt[:], attn[:, b * 128:b * 128 + 128],
                                             id16[:])
                         if b == 0:
                             nc.vector.tensor_copy(attnT[:, b, :], ptt[:])
                         else:
                             nc.scalar.activation(
                                 attnT[:, b, :], ptt[:],
                                 mybir.ActivationFunctionType.Copy)
                     po = ps_o.tile([128, 64], F32)
                     for b in range(2):
                         nc.tensor.matmul(po[:], attnT[:, b, :],
                                          v16[:, it + b, :],
                                          start=(b == 0), stop=(b == 1))
                     recip = smp.tile([128, 1], F32, tag="recip")
                     nc.vector.reciprocal(recip[:], rowsum[:])
                     nc.vector.tensor_scalar_mul(o16[:, h, :], po[:], recip[:])

                 # oT: transpose o16 [128, 1024] -> 8 chunks [128, 128]
                 for c in range(8):
                     pt = ps_t.tile([128, 128], F16, tag="tp")
                     nc.tensor.transpose(
                         pt[:], o16[:, 2 * c:2 * c + 2, :].rearrange("p a b -> p (a b)"),
                         id16[:])
                     nc.vector.tensor_copy(oT_sb[:, c, it * 128:(it + 1) * 128], pt[:])

             # outproj over both itiles at once (N=256)
             for nt in ([] if abl == "noout" else range(8)):
                 pf = ps_f.tile([128, 256], F32)
                 for c in range(8):
                     nc.tensor.matmul(pf[:], w2[:, c, 128 * nt:128 * (nt + 1)],
                                      oT_sb[:, c, :],
                                      start=(c == 0), stop=(c == 7))
                 ot = outp.tile([128, 256], F32, tag="ot")
                 nc.scalar.activation(ot[:], pf[:],
                                      mybir.ActivationFunctionType.Identity,
                                      bias=b2[:, nt:nt + 1], scale=1.0)
                 nc.sync.dma_start(out_d[nt], ot[:])

    nc.compile()
    return nc


def _prep_inputs(x, Wqkv, bqkv, Wout, bout):
    x = np.asarray(x, dtype=np.float32)
    Wqkv = np.asarray(Wqkv, dtype=np.float32)
    bqkv = np.asarray(bqkv, dtype=np.float32)
    Wout = np.asarray(Wout, dtype=np.float32)
    bout = np.asarray(bout, dtype=np.float32)

    sq = np.sqrt(np.float32(D))
    W1 = Wqkv.copy()
    b1 = bqkv.copy()
    W1[2 * D:] *= sq
    b1[2 * D:] *= sq
    w1T = _rnd_fp32r(np.ascontiguousarray(W1.T)).reshape(8, 128, 1152)
    b1t = np.ascontiguousarray(b1.reshape(9, 128).T)          # [128, 9]
    w2T = np.ascontiguousarray(Wout.T).astype(np.float16).reshape(8, 128, 1024)
    b2t = np.ascontiguousarray(bout.reshape(8, 128).T)        # [128, 8]

    pi = np.arange(128)[:, None]
    fj = np.arange(256)[None, :]
    std = np.where((fj > pi) & (fj <= pi + 128), 0.0, -60000.0).astype(np.float16)
    edge = np.where((fj > pi) & (fj <= pi + 128) & (fj >= 128), 0.0,
                    -60000.0).astype(np.float16)
    ident = np.eye(128, dtype=np.float16)

    in_maps = []
    for c in range(NC):
        r0 = c * RPC
        xs = np.zeros((XW, DM), np.float32)
        lo = max(0, r0 - HALO)
        xs[HALO - (r0 - lo):HALO + RPC] = x[lo:r0 + RPC]
        xTc = _rnd_fp32r(np.ascontiguousarray(xs.T)).reshape(8, 128, XW)
        m0 = edge if c == 0 else std
        mc = np.ascontiguousarray(
            np.stack([np.concatenate([m0, m0], 1),
                      np.concatenate([std, std], 1)], axis=1))  # [128, 2, 512]
        in_maps.append({
            "xT": xTc, "w1T": w1T, "b1": b1t, "w2T": w2T, "b2": b2t,
            "mask": mc, "ident": ident, "ident16": np.eye(128, dtype=np.float16),
        })
    return in_maps


def kernel(x, Wqkv, bqkv, Wout, bout):
    if "nc" not in _CACHED:
        _CACHED["nc"] = _build()
    nc = _CACHED["nc"]
    in_maps = _prep_inputs(x, Wqkv, bqkv, Wout, bout)
    res = run_bass_kernel_spmd(nc, in_maps, list(range(NC)))
    out = np.empty((S, DM), np.float32)
    for c in range(NC):
        outT = res.results[c]["outT"].reshape(DM, RPC)
        out[c * RPC:(c + 1) * RPC] = outT.T
    return out


if __name__ == "__main__":
    rng = np.random.default_rng(0)
    ins = {
        "x": rng.standard_normal((S, DM)).astype(np.float32),
        "Wqkv": (rng.standard_normal((1152, DM)) / 32).astype(np.float32),
        "bqkv": (rng.standard_normal((1152,)) * 0.01).astype(np.float32),
        "Wout": (rng.standard_normal((DM, DM)) / 32).astype(np.float32),
        "bout": (rng.standard_normal((DM,)) * 0.01).astype(np.float32),
    }
    out = kernel(**ins)
    print("kernel ran, out shape", out.shape)



# revision 10
# speedup vs baseline: 1.1889x; 1.1889x over previous
"""Trainium2 Bass kernel for nn_LocalMQA (S=2048, D_MODEL=1024, H=16, D=64, WIN=128).

Sharding: sequence-parallel across 8 cores (256 output rows each) with a
128-row halo recomputed for k/v. No collectives; each core produces a
disjoint slice of the output.

Per-core pipeline, all fp16 matmuls (PSUM accumulates f32):
  qkvT = W1T.T @ xT              (fp16, q pre-scaled by sqrt(D) on host)
  scores = maskinject + qT.k     (mask via fp16 identity matmul into PSUM)
  softmax: DVE rowmax(negate) -> ACT Exp(bias=-max) -> fp16 attn
  attnT via PE transpose; po = attnT.T @ [v16|1]  (ones col gives rowsum Z)
  o16 = po[:,0:64] * (1/Z)  (recip on DVE, scale-evac on ACT/DVE)
  oT via PE transpose; outT = W2T.T @ oT + bout -> DRAM fp16 [1024, 256]
Host transposes/concats/casts the 8 outT slices into the final (2048, 1024).
"""
import contextlib

import numpy as np

import concourse.bacc as bacc
import concourse.mybir as mybir
import concourse.tile as tile
from concourse.bass_utils import run_bass_kernel_spmd

S = 2048
DM = 1024
H = 16
D = 64
WIN = 128
NC = 8
RPC = S // NC          # rows per core = 256
HALO = 128
XW = RPC + HALO        # per-core xT width = 384

F32 = mybir.dt.float32
F16 = mybir.dt.float16

_CACHED = {}


def _nullctx():
    return contextlib.nullcontext()


def _build(debug=False, reps=None, abl=None):
    nc = bacc.Bacc("TRN2", target_bir_lowering=False, debug=False, num_devices=NC)

    xT_d = nc.dram_tensor("xT", [8, 128, XW], F16, kind="ExternalInput").ap()
    w1_d = nc.dram_tensor("w1T", [8, 128, 1152], F16, kind="ExternalInput").ap()
    b1_d = nc.dram_tensor("b1", [128, 9], F32, kind="ExternalInput").ap()
    w2_d = nc.dram_tensor("w2T", [8, 128, 1024], F16, kind="ExternalInput").ap()
    b2_d = nc.dram_tensor("b2", [128, 8], F32, kind="ExternalInput").ap()
    msk_d = nc.dram_tensor("mask", [128, 2, 512], F16, kind="ExternalInput").ap()
    id16_d = nc.dram_tensor("ident16", [128, 128], F16, kind="ExternalInput").ap()
    out_d = nc.dram_tensor("outT", [8, 128, RPC], F16, kind="ExternalOutput").ap()

    AF = mybir.ActivationFunctionType
    if debug:
        dbg = {
            "dbg_kv": nc.dram_tensor("dbg_kv", [128, XW], F16, kind="ExternalOutput").ap(),
            "dbg_q": nc.dram_tensor("dbg_q", [128, 8, RPC], F16, kind="ExternalOutput").ap(),
            "dbg_v16": nc.dram_tensor("dbg_v16", [128, 3, 65], F16, kind="ExternalOutput").ap(),
            "dbg_negm": nc.dram_tensor("dbg_negm", [128, 32], F32, kind="ExternalOutput").ap(),
            "dbg_attn": nc.dram_tensor("dbg_attn", [128, 256], F16, kind="ExternalOutput").ap(),
            "dbg_attnT": nc.dram_tensor("dbg_attnT", [128, 2, 128], F16, kind="ExternalOutput").ap(),
            "dbg_po": nc.dram_tensor("dbg_po", [128, 65], F32, kind="ExternalOutput").ap(),
            "dbg_o16": nc.dram_tensor("dbg_o16", [2, 128, 16, 64], F16, kind="ExternalOutput").ap(),
            "dbg_oT": nc.dram_tensor("dbg_oT", [128, 8, 256], F16, kind="ExternalOutput").ap(),
        }

    with tile.TileContext(nc) as tc:
      with (
        tc.tile_pool(name="w", bufs=1) as wp,      # weights + constants
        tc.tile_pool(name="act", bufs=1) as ap_,   # persistent activations
        tc.tile_pool(name="sm", bufs=8) as smp,    # small softmax tiles
        tc.tile_pool(name="att", bufs=6) as attp,
        tc.tile_pool(name="o16p", bufs=1) as o16p,
        tc.tile_pool(name="outp", bufs=2) as outp,
      ):
       with (tc.For_i(0, reps, 1) if reps else _nullctx()):
         with (
             tc.tile_pool(name="ps_q", bufs=2, space="PSUM") as ps_q,
             tc.tile_pool(name="ps_kv", bufs=1, space="PSUM") as ps_kv,
         ):
             # ---- input DMAs ----
             xT = wp.tile([128, 8, XW], F16)
             w1 = wp.tile([128, 8, 1152], F16)
             w2 = wp.tile([128, 8, 1024], F16)
             b1 = wp.tile([128, 9], F32)
             b2 = wp.tile([128, 8], F32)
             msk2 = wp.tile([128, 2, 512], F16)
             id16 = wp.tile([128, 128], F16)
             nc.sync.dma_start(id16[:], id16_d)
             nc.sync.dma_start(b1[:], b1_d)
             nc.sync.dma_start(msk2[:], msk_d)
             for c in range(8):
                 nc.sync.dma_start(xT[:, c, :], xT_d[c])
                 nc.sync.dma_start(w1[:, c, :], w1_d[c])
             nc.sync.dma_start(b2[:], b2_d)
             for c in range(8):
                 nc.sync.dma_start(w2[:, c, :], w2_d[c])

             # ---- qkv projection: qkvT tiles [outdim-part, rows-free] ----
             kv_sb = ap_.tile([128, XW], F16)       # k rows 0:64 (+ mirror)
             v16r = ap_.tile([128, 3, 128], F16)    # v (cast) at partitions 64:128
             q_sb = ap_.tile([128, 8, RPC], F16)    # q tiles, 2 heads per tile

             kvp = ps_kv.tile([128, XW], F32)
             for c in range(8):
                 nc.tensor.matmul(kvp[:], w1[:, c, 0:128], xT[:, c, :],
                                  start=(c == 0), stop=(c == 7))
             # k evac on ACT, v evac on DVE
             nc.scalar.activation(kv_sb[0:64, :], kvp[0:64, :],
                                  AF.Identity, bias=b1[0:64, 0:1], scale=1.0)
             nc.vector.tensor_scalar_add(
                 v16r[64:128, :, :].rearrange("p b n -> p (b n)"), kvp[64:128, :],
                 b1[64:128, 0:1])

             for t in range(8):
                 qp = ps_q.tile([128, RPC], F32)
                 for c in range(8):
                     nc.tensor.matmul(
                         qp[:], w1[:, c, 128 * (t + 1):128 * (t + 2)],
                         xT[:, c, HALO:XW],
                         start=(c == 0), stop=(c == 7))
                 nc.vector.tensor_scalar_add(q_sb[:, t, :], qp[:], b1[:, t + 1:t + 2])

             # v16e: transpose v [64, 384] -> 3 blocks [128, 64] + ones col 64
             v16t = ap_.tile([128, 3, 64], F16)
             v16 = ap_.tile([128, 3, 65], F16)
             nc.vector.memset(v16[:, :, 64:65], 1.0)
             for b in range(3):
                 nc.sync.dma_start(v16t[:, b, :], v16r[64:128, b, :], transpose=True)
             nc.vector.tensor_copy(v16[:, :, 0:64], v16t[:])
             # mirror k into partitions 64:128 so odd heads (q at base
             # partition 64) have a same-base rhs (matmul requirement)
             nc.sync.dma_start(kv_sb[64:128, :], kv_sb[0:64, :])
             if debug:
                 nc.sync.dma_start(dbg["dbg_kv"], kv_sb[:])
                 nc.sync.dma_start(dbg["dbg_q"], q_sb[:])
                 nc.sync.dma_start(dbg["dbg_v16"], v16[:])

         with (
             tc.tile_pool(name="ps_s", bufs=3, space="PSUM") as ps_s,
             tc.tile_pool(name="ps_o", bufs=2, space="PSUM") as ps_o,
             tc.tile_pool(name="ps_t", bufs=2, space="PSUM") as ps_t,
             tc.tile_pool(name="ps_f", bufs=1, space="PSUM") as ps_f,
         ):
             oT_sb = o16p.tile([128, 8, 2 * 128], F16)  # [hd-chunk, itile*128+i]
             if debug:
                 negm_all = o16p.tile([128, 32], F32, tag="negm_all")
             for it in range(2):
                 o16 = o16p.tile([128, 16, 64], F16, tag=f"o16_{it}")
                 for h in range(16):
                     sc = ps_s.tile([128, 256], F32)
                     nc.tensor.matmul(sc[:], id16[:],
                                      msk2[:, min(it, 1), 0:256],
                                      start=True, stop=False)
                     nc.tensor.matmul(
                         sc[:],
                         q_sb[64 * (h % 2):64 * (h % 2) + 64, h // 2,
                              it * 128:it * 128 + 128],
                         kv_sb[64 * (h % 2):64 * (h % 2) + 64,
                               it * 128:it * 128 + 256],
                         start=False, stop=True)
                     negm = smp.tile([128, 1], F32, tag="negm")
                     nc.vector.tensor_reduce(negm[:], sc[:],
                                             axis=mybir.AxisListType.X,
                                             op=mybir.AluOpType.max, negate=True)
                     if debug:
                         nc.vector.tensor_copy(negm_all[:, 16 * it + h:16 * it + h + 1],
                                               negm[:])
                     attn = attp.tile([128, 256], F16, tag="attn")
                     nc.scalar.activation(attn[:], sc[:], AF.Exp,
                                          bias=negm[:], scale=1.0)
                     if debug and h == 0 and it == 0:
                         nc.sync.dma_start(dbg["dbg_attn"], attn[:])
                     ptt = ps_t.tile([128, 2, 128], F16, tag="tp")
                     for b in range(2):
                         nc.tensor.transpose(ptt[:, b, :],
                                             attn[:, b * 128:b * 128 + 128],
                                             id16[:])
                     attnT = attp.tile([128, 2, 128], F16, tag="attnT")
                     if h % 2 == 0:
                         nc.vector.tensor_copy(
                             attnT[:].rearrange("p a b -> p (a b)"),
                             ptt[:].rearrange("p a b -> p (a b)"))
                     else:
                         nc.scalar.activation(
                             attnT[:].rearrange("p a b -> p (a b)"),
                             ptt[:].rearrange("p a b -> p (a b)"), AF.Copy)
                     po = ps_o.tile([128, 65], F32)
                     for b in range(2):
                         nc.tensor.matmul(po[:], attnT[:, b, :],
                                          v16[:, it + b, :],
                                          start=(b == 0), stop=(b == 1))
                     if debug and h == 0 and it == 0:
                         nc.sync.dma_start(dbg["dbg_attnT"], attnT[:])
                         dbg_po_sb = o16p.tile([128, 65], F32, tag="dbg_po_sb")
                         nc.vector.tensor_copy(dbg_po_sb[:], po[:])
                         nc.sync.dma_start(dbg["dbg_po"], dbg_po_sb[:])
                     recip = smp.tile([128, 1], F32, tag="recip")
                     nc.vector.reciprocal(recip[:], po[:, 64:65])
                     if h % 2 == 0:
                         nc.scalar.activation(o16[:, h, :], po[:, 0:64],
                                              AF.Copy, scale=recip[:])
                     else:
                         nc.vector.tensor_scalar_mul(o16[:, h, :], po[:, 0:64],
                                                     recip[:])

                 if debug:
                     nc.sync.dma_start(dbg["dbg_o16"][it], o16[:])
                 # oT: transpose o16 [128, 1024] -> 8 chunks [128, 128]
                 for c in range(8):
                     pt = ps_t.tile([128, 2, 128], F16, tag="tp")
                     nc.tensor.transpose(
                         pt[:, 0, :],
                         o16[:, 2 * c:2 * c + 2, :].rearrange("p a b -> p (a b)"),
                         id16[:])
                     if c % 2 == 0:
                         nc.vector.tensor_copy(oT_sb[:, c, it * 128:(it + 1) * 128],
                                               pt[:, 0, :])
                     else:
                         nc.scalar.activation(oT_sb[:, c, it * 128:(it + 1) * 128],
                                              pt[:, 0, :], AF.Copy)

             if debug:
                 nc.sync.dma_start(dbg["dbg_oT"], oT_sb[:])
                 nc.sync.dma_start(dbg["dbg_negm"], negm_all[:])
             # outproj over both itiles at once (N=256)
             for nt in range(8):
                 pf = ps_f.tile([128, 256], F32)
                 for c in range(8):
                     nc.tensor.matmul(pf[:], w2[:, c, 128 * nt:128 * (nt + 1)],
                                      oT_sb[:, c, :],
                                      start=(c == 0), stop=(c == 7))
                 ot = outp.tile([128, 256], F16, tag="ot")
                 nc.scalar.activation(ot[:], pf[:], AF.Identity,
                                      bias=b2[:, nt:nt + 1], scale=1.0)
                 nc.sync.dma_start(out_d[nt], ot[:])

    nc.compile()
    return nc


def _prep_inputs(x, Wqkv, bqkv, Wout, bout):
    x = np.asarray(x, dtype=np.float32)
    Wqkv = np.asarray(Wqkv, dtype=np.float32)
    bqkv = np.asarray(bqkv, dtype=np.float32)
    Wout = np.asarray(Wout, dtype=np.float32)
    bout = np.asarray(bout, dtype=np.float32)

    sq = np.sqrt(np.float32(D))
    W1 = Wqkv.copy()
    b1 = bqkv.copy()
    W1[2 * D:] *= sq
    b1[2 * D:] *= sq
    w1T = np.ascontiguousarray(W1.T).astype(np.float16).reshape(8, 128, 1152)
    b1t = np.ascontiguousarray(b1.reshape(9, 128).T)          # [128, 9]
    w2T = np.ascontiguousarray(Wout.T).astype(np.float16).reshape(8, 128, 1024)
    b2t = np.ascontiguousarray(bout.reshape(8, 128).T)        # [128, 8]

    pi = np.arange(128)[:, None]
    fj = np.arange(256)[None, :]
    std = np.where((fj > pi) & (fj <= pi + 128), 0.0, -60000.0).astype(np.float16)
    edge = np.where((fj > pi) & (fj <= pi + 128) & (fj >= 128), 0.0,
                    -60000.0).astype(np.float16)
    ident = np.eye(128, dtype=np.float16)

    in_maps = []
    for c in range(NC):
        r0 = c * RPC
        xs = np.zeros((XW, DM), np.float32)
        lo = max(0, r0 - HALO)
        xs[HALO - (r0 - lo):HALO + RPC] = x[lo:r0 + RPC]
        xTc = np.ascontiguousarray(xs.T).astype(np.float16).reshape(8, 128, XW)
        m0 = edge if c == 0 else std
        mc = np.ascontiguousarray(
            np.stack([np.concatenate([m0, m0], 1),
                      np.concatenate([std, std], 1)], axis=1))  # [128, 2, 512]
        in_maps.append({
            "xT": xTc, "w1T": w1T, "b1": b1t, "w2T": w2T, "b2": b2t,
            "mask": mc, "ident16": ident,
        })
    return in_maps


def kernel(x, Wqkv, bqkv, Wout, bout):
    if "nc" not in _CACHED:
        _CACHED["nc"] = _build()
    nc = _CACHED["nc"]
    in_maps = _prep_inputs(x, Wqkv, bqkv, Wout, bout)
    res = run_bass_kernel_spmd(nc, in_maps, list(range(NC)))
    out = np.empty((S, DM), np.float32)
    for c in range(NC):
        outT = res.results[c]["outT"].reshape(DM, RPC)
        out[c * RPC:(c + 1) * RPC] = outT.T.astype(np.float32)
    return out


if __name__ == "__main__":
    rng = np.random.default_rng(0)
    ins = {
        "x": rng.standard_normal((S, DM)).astype(np.float32),
        "Wqkv": (rng.standard_normal((1152, DM)) / 32).astype(np.float32),
        "bqkv": (rng.standard_normal((1152,)) * 0.01).astype(np.float32),
        "Wout": (rng.standard_normal((DM, DM)) / 32).astype(np.float32),
        "bout": (rng.standard_normal((DM,)) * 0.01).astype(np.float32),
    }
    out = kernel(**ins)
    print("kernel ran, out shape", out.shape)


# revision 30
# speedup vs baseline: 1.3301x; 1.1188x over previous
"""Trainium2 Bass kernel for nn_LocalMQA (S=2048, D_MODEL=1024, H=16, D=64, WIN=128).

Sharding: sequence-parallel across 8 cores (256 output rows each) with a
128-row halo recomputed for k/v. No collectives; each core produces a
disjoint slice of the output.

Per-core pipeline, all fp16 matmuls (PSUM accumulates f32):
  qkvT = W1T.T @ xT              (fp16, q pre-scaled by sqrt(D) on host)
  scores = maskinject + qT.k     (mask via fp16 identity matmul into PSUM)
  softmax: DVE rowmax(negate) -> ACT Exp(bias=-max) -> fp16 attn
  attnT via PE transpose; po = attnT.T @ [v16|1]  (ones col gives rowsum Z)
  o16 = po[:,0:64] * (1/Z)  (recip on DVE, scale-evac on ACT/DVE)
  oT via PE transpose; outT = W2T.T @ oT + bout -> DRAM fp16 [1024, 256]
Host transposes/concats/casts the 8 outT slices into the final (2048, 1024).
"""
import contextlib

import numpy as np

import concourse.bacc as bacc
import concourse.mybir as mybir
import concourse.tile as tile
from concourse.bass_utils import run_bass_kernel_spmd

S = 2048
DM = 1024
H = 16
D = 64
WIN = 128
NC = 8
RPC = S // NC          # rows per core = 256
HALO = 128
XW = RPC + HALO        # per-core xT width = 384

F32 = mybir.dt.float32
F16 = mybir.dt.float16

_CACHED = {}
import os as _os
NEW_DMA = [_os.environ.get("K_NEW_DMA") == "1"]


def _nullctx():
    return contextlib.nullcontext()


def _build(debug=False, reps=None, abl=None):
    nc = bacc.Bacc("TRN2", target_bir_lowering=False, debug=False, num_devices=NC)

    xT_d = nc.dram_tensor("xT", [8, 128, XW], F16, kind="ExternalInput").ap()
    w1_d = nc.dram_tensor("w1T", [8, 128, 1152], F16, kind="ExternalInput").ap()
    b1_d = nc.dram_tensor("b1", [128, 9], F32, kind="ExternalInput").ap()
    w2_d = nc.dram_tensor("w2T", [8, 128, 1024], F16, kind="ExternalInput").ap()
    b2_d = nc.dram_tensor("b2", [128, 8], F32, kind="ExternalInput").ap()
    msk_d = nc.dram_tensor("mask", [128, 2, 512], F16, kind="ExternalInput").ap()
    id16_d = nc.dram_tensor("ident16", [128, 128], F16, kind="ExternalInput").ap()
    out_d = nc.dram_tensor("outT", [8, 128, RPC], F16, kind="ExternalOutput").ap()

    AF = mybir.ActivationFunctionType
    if debug:
        dbg = {
            "dbg_kv": nc.dram_tensor("dbg_kv", [128, XW], F16, kind="ExternalOutput").ap(),
            "dbg_q": nc.dram_tensor("dbg_q", [128, 8, RPC], F16, kind="ExternalOutput").ap(),
            "dbg_v16": nc.dram_tensor("dbg_v16", [128, 3, 65], F16, kind="ExternalOutput").ap(),
            "dbg_negm": nc.dram_tensor("dbg_negm", [128, 32], F32, kind="ExternalOutput").ap(),
            "dbg_attn": nc.dram_tensor("dbg_attn", [128, 256], F16, kind="ExternalOutput").ap(),
            "dbg_attnT": nc.dram_tensor("dbg_attnT", [128, 2, 128], F16, kind="ExternalOutput").ap(),
            "dbg_po": nc.dram_tensor("dbg_po", [128, 65], F32, kind="ExternalOutput").ap(),
            "dbg_o16": nc.dram_tensor("dbg_o16", [2, 128, 16, 64], F16, kind="ExternalOutput").ap(),
            "dbg_oT": nc.dram_tensor("dbg_oT", [128, 8, 256], F16, kind="ExternalOutput").ap(),
        }

    with tile.TileContext(nc) as tc:
      with (
        tc.tile_pool(name="w", bufs=1) as wp,      # weights + constants
        tc.tile_pool(name="act", bufs=1) as ap_,   # persistent activations
        tc.tile_pool(name="sm", bufs=8) as smp,    # small softmax tiles
        tc.tile_pool(name="att", bufs=6) as attp,
        tc.tile_pool(name="o16p", bufs=1) as o16p,
        tc.tile_pool(name="outp", bufs=2) as outp,
      ):
       with (tc.For_i(0, reps, 1) if reps else _nullctx()):
         with (
             tc.tile_pool(name="ps_q", bufs=2, space="PSUM") as ps_q,
             tc.tile_pool(name="ps_kv", bufs=1, space="PSUM") as ps_kv,
         ):
             # ---- input DMAs (batched; issue on idle queues) ----
             xT = wp.tile([128, 8, XW], F16)
             w1 = wp.tile([128, 8, 1152], F16)
             w2 = wp.tile([128, 8, 1024], F16)
             b1 = wp.tile([128, 9], F32)
             b2 = wp.tile([128, 8], F32)
             msk2 = wp.tile([128, 2, 512], F16)
             id16 = wp.tile([128, 128], F16)
             if NEW_DMA[0]:
                 nc.gpsimd.dma_start(id16[:], id16_d)
                 nc.gpsimd.dma_start(b1[:], b1_d)
                 nc.gpsimd.dma_start(msk2[:], msk_d)
                 nc.gpsimd.dma_start(b2[:], b2_d)
                 nc.sync.dma_start(xT[:], xT_d.rearrange("c p n -> p c n"))
                 for c in range(4):
                     nc.sync.dma_start(
                         w1[:, 2 * c:2 * c + 2, :],
                         w1_d[2 * c:2 * c + 2].rearrange("c p n -> p c n"))
                 nc.scalar.dma_start(w2[:], w2_d.rearrange("c p n -> p c n"))
             else:
                 nc.sync.dma_start(id16[:], id16_d)
                 nc.sync.dma_start(b1[:], b1_d)
                 nc.sync.dma_start(msk2[:], msk_d)
                 for c in range(8):
                     nc.sync.dma_start(xT[:, c, :], xT_d[c])
                     nc.sync.dma_start(w1[:, c, :], w1_d[c])
                 nc.sync.dma_start(b2[:], b2_d)
                 for c in range(8):
                     nc.sync.dma_start(w2[:, c, :], w2_d[c])

             # ---- qkv projection: qkvT tiles [outdim-part, rows-free] ----
             kv_sb = ap_.tile([128, XW], F16)       # k rows 0:64 (+ mirror)
             v16r = ap_.tile([128, 3, 128], F16)    # v (cast) at partitions 64:128
             q_sb = ap_.tile([128, 8, RPC], F16)    # q tiles, 2 heads per tile

             kvp = ps_kv.tile([128, XW], F32)
             for c in range(8):
                 nc.tensor.matmul(kvp[:], w1[:, c, 0:128], xT[:, c, :],
                                  start=(c == 0), stop=(c == 7))
             # k evac on ACT, v evac on DVE
             nc.scalar.activation(kv_sb[0:64, :], kvp[0:64, :],
                                  AF.Identity, bias=b1[0:64, 0:1], scale=1.0)
             nc.vector.tensor_scalar_add(
                 v16r[64:128, :, :].rearrange("p b n -> p (b n)"), kvp[64:128, :],
                 b1[64:128, 0:1])

             for t in range(8):
                 qp = ps_q.tile([128, RPC], F32)
                 for c in range(8):
                     nc.tensor.matmul(
                         qp[:], w1[:, c, 128 * (t + 1):128 * (t + 2)],
                         xT[:, c, HALO:XW],
                         start=(c == 0), stop=(c == 7))
                 nc.vector.tensor_scalar_add(q_sb[:, t, :], qp[:], b1[:, t + 1:t + 2])

             # v16e: transpose v [64, 384] -> 3 blocks [128, 64] + ones col 64
             v16t = ap_.tile([128, 3, 64], F16)
             v16 = ap_.tile([128, 3, 65], F16)
             nc.vector.memset(v16[:, :, 64:65], 1.0)
             for b in range(3):
                 nc.sync.dma_start(v16t[:, b, :], v16r[64:128, b, :], transpose=True)
             nc.vector.tensor_copy(v16[:, :, 0:64], v16t[:])
             # mirror k into partitions 64:128 so odd heads (q at base
             # partition 64) have a same-base rhs (matmul requirement)
             nc.sync.dma_start(kv_sb[64:128, :], kv_sb[0:64, :])
             if debug:
                 nc.sync.dma_start(dbg["dbg_kv"], kv_sb[:])
                 nc.sync.dma_start(dbg["dbg_q"], q_sb[:])
                 nc.sync.dma_start(dbg["dbg_v16"], v16[:])

         with (
             tc.tile_pool(name="ps_s", bufs=3, space="PSUM") as ps_s,
             tc.tile_pool(name="ps_o", bufs=2, space="PSUM") as ps_o,
             tc.tile_pool(name="ps_t", bufs=2, space="PSUM") as ps_t,
             tc.tile_pool(name="ps_f", bufs=1, space="PSUM") as ps_f,
         ):
             oT_sb = o16p.tile([128, 8, 2 * 128], F16)  # [hd-chunk, itile*128+i]
             if debug:
                 negm_all = o16p.tile([128, 32], F32, tag="negm_all")
             for it in range(2):
                 o16 = o16p.tile([128, 16, 64], F16, tag=f"o16_{it}")
                 for h in range(16):
                     sc = ps_s.tile([128, 256], F32, tag="sc")
                     nc.tensor.matmul(sc[:], id16[:],
                                      msk2[:, min(it, 1), 0:256],
                                      start=True, stop=False)
                     nc.tensor.matmul(
                         sc[:],
                         q_sb[64 * (h % 2):64 * (h % 2) + 64, h // 2,
                              it * 128:it * 128 + 128],
                         kv_sb[64 * (h % 2):64 * (h % 2) + 64,
                               it * 128:it * 128 + 256],
                         start=False, stop=True)
                     negm = smp.tile([128, 1], F32, tag="negm")
                     nc.vector.tensor_reduce(negm[:], sc[:],
                                             axis=mybir.AxisListType.X,
                                             op=mybir.AluOpType.max, negate=True)
                     if debug:
                         nc.vector.tensor_copy(
                             negm_all[:, 16 * it + h:16 * it + h + 1], negm[:])
                     attn = attp.tile([128, 256], F16, tag="attn")
                     nc.scalar.activation(attn[:], sc[:], AF.Exp,
                                          bias=negm[:], scale=1.0)
                     if debug and h == 0 and it == 0:
                         nc.sync.dma_start(dbg["dbg_attn"], attn[:])
                     ptt = ps_t.tile([128, 2, 128], F16, tag="tp")
                     for b in range(2):
                         nc.tensor.transpose(ptt[:, b, :],
                                             attn[:, b * 128:b * 128 + 128],
                                             id16[:])
                     attnT = attp.tile([128, 2, 128], F16, tag="attnT")
                     if h % 2 == 0:
                         nc.vector.tensor_copy(
                             attnT[:].rearrange("p a b -> p (a b)"),
                             ptt[:].rearrange("p a b -> p (a b)"))
                     else:
                         nc.scalar.activation(
                             attnT[:].rearrange("p a b -> p (a b)"),
                             ptt[:].rearrange("p a b -> p (a b)"), AF.Copy)
                     po = ps_o.tile([128, 65], F32, tag="po")
                     for b in range(2):
                         nc.tensor.matmul(po[:], attnT[:, b, :],
                                          v16[:, it + b, :],
                                          start=(b == 0), stop=(b == 1))
                     if debug and h == 0 and it == 0:
                         nc.sync.dma_start(dbg["dbg_attnT"], attnT[:])
                         dbg_po_sb = o16p.tile([128, 65], F32, tag="dbg_po_sb")
                         nc.vector.tensor_copy(dbg_po_sb[:], po[:])
                         nc.sync.dma_start(dbg["dbg_po"], dbg_po_sb[:])
                     recip = smp.tile([128, 1], F32, tag="recip")
                     nc.vector.reciprocal(recip[:], po[:, 64:65])
                     if h % 2 == 0:
                         nc.scalar.activation(o16[:, h, :], po[:, 0:64],
                                              AF.Copy, scale=recip[:])
                     else:
                         nc.vector.tensor_scalar_mul(o16[:, h, :],
                                                     po[:, 0:64], recip[:])

                 if debug:
                     nc.sync.dma_start(dbg["dbg_o16"][it], o16[:])
                 # oT: transpose o16 [128, 1024] -> 8 chunks [128, 128]
                 for c in range(8):
                     pt = ps_t.tile([128, 2, 128], F16, tag="tp")
                     nc.tensor.transpose(
                         pt[:, 0, :],
                         o16[:, 2 * c:2 * c + 2, :].rearrange("p a b -> p (a b)"),
                         id16[:])
                     if c % 2 == 0:
                         nc.scalar.activation(oT_sb[:, c, it * 128:(it + 1) * 128],
                                              pt[:, 0, :], AF.Copy)
                     else:
                         nc.vector.tensor_copy(oT_sb[:, c, it * 128:(it + 1) * 128],
                                               pt[:, 0, :])

             if debug:
                 nc.sync.dma_start(dbg["dbg_oT"], oT_sb[:])
                 nc.sync.dma_start(dbg["dbg_negm"], negm_all[:])
             # outproj over both itiles at once (N=256)
             for nt in range(8):
                 pf = ps_f.tile([128, 256], F32)
                 for c in range(8):
                     nc.tensor.matmul(pf[:], w2[:, c, 128 * nt:128 * (nt + 1)],
                                      oT_sb[:, c, :],
                                      start=(c == 0), stop=(c == 7))
                 ot = outp.tile([128, 256], F16, tag="ot")
                 nc.scalar.activation(ot[:], pf[:], AF.Identity,
                                      bias=b2[:, nt:nt + 1], scale=1.0)
                 nc.sync.dma_start(out_d[nt], ot[:])

    nc.compile()
    return nc


def _prep_inputs(x, Wqkv, bqkv, Wout, bout):
    x = np.asarray(x, dtype=np.float32)
    Wqkv = np.asarray(Wqkv, dtype=np.float32)
    bqkv = np.asarray(bqkv, dtype=np.float32)
    Wout = np.asarray(Wout, dtype=np.float32)
    bout = np.asarray(bout, dtype=np.float32)

    sq = np.sqrt(np.float32(D))
    W1 = Wqkv.copy()
    b1 = bqkv.copy()
    W1[2 * D:] *= sq
    b1[2 * D:] *= sq
    w1T = np.ascontiguousarray(W1.T).astype(np.float16).reshape(8, 128, 1152)
    b1t = np.ascontiguousarray(b1.reshape(9, 128).T)          # [128, 9]
    w2T = np.ascontiguousarray(Wout.T).astype(np.float16).reshape(8, 128, 1024)
    b2t = np.ascontiguousarray(bout.reshape(8, 128).T)        # [128, 8]

    pi = np.arange(128)[:, None]
    fj = np.arange(256)[None, :]
    std = np.where((fj > pi) & (fj <= pi + 128), 0.0, -60000.0).astype(np.float16)
    edge = np.where((fj > pi) & (fj <= pi + 128) & (fj >= 128), 0.0,
                    -60000.0).astype(np.float16)
    ident = np.eye(128, dtype=np.float16)

    in_maps = []
    for c in range(NC):
        r0 = c * RPC
        xs = np.zeros((XW, DM), np.float32)
        lo = max(0, r0 - HALO)
        xs[HALO - (r0 - lo):HALO + RPC] = x[lo:r0 + RPC]
        xTc = np.ascontiguousarray(xs.T).astype(np.float16).reshape(8, 128, XW)
        m0 = edge if c == 0 else std
        mc = np.ascontiguousarray(
            np.stack([np.concatenate([m0, m0], 1),
                      np.concatenate([std, std], 1)], axis=1))  # [128, 2, 512]
        in_maps.append({
            "xT": xTc, "w1T": w1T, "b1": b1t, "w2T": w2T, "b2": b2t,
            "mask": mc, "ident16": ident,
        })
    return in_maps


def kernel(x, Wqkv, bqkv, Wout, bout):
    if "nc" not in _CACHED:
        _CACHED["nc"] = _build()
    nc = _CACHED["nc"]
    in_maps = _prep_inputs(x, Wqkv, bqkv, Wout, bout)
    res = run_bass_kernel_spmd(nc, in_maps, list(range(NC)))
    out = np.empty((S, DM), np.float32)
    for c in range(NC):
        outT = res.results[c]["outT"].reshape(DM, RPC)
        out[c * RPC:(c + 1) * RPC] = outT.T.astype(np.float32)
    return out


if __name__ == "__main__":
    rng = np.random.default_rng(0)
    ins = {
        "x": rng.standard_normal((S, DM)).astype(np.float32),
        "Wqkv": (rng.standard_normal((1152, DM)) / 32).astype(np.float32),
        "bqkv": (rng.standard_normal((1152,)) * 0.01).astype(np.float32),
        "Wout": (rng.standard_normal((DM, DM)) / 32).astype(np.float32),
        "bout": (rng.standard_normal((DM,)) * 0.01).astype(np.float32),
    }
    out = kernel(**ins)
    print("kernel ran, out shape", out.shape)
